# revision 1
# baseline (speedup 1.0000x reference)
"""Trainium2 Bass kernel for DiffusionGraphConv (Chebyshev K=2 graph conv).

v2 strategy (8 NeuronCores, pair-shared DRAM, bf16 gathers):
  - reference: x0 = [N, F*B] (col = f*B + b), x1 = S@x0, x2 = 2*S@x1 - x0,
    out[b*N+n, o] = sum_{f,m} x_m[n, col(f,b)] W[f*M+m, o] + bias.
  - cores are grouped in LNC2 pairs (2q, 2q+1) that share DRAM scratch.
    Pair q owns batches 4q..4q+3 (C2=256 reordered cols, col = b'*64 + f);
    within the pair, core h = c%2 owns dst rows [h*10000, (h+1)*10000).
  - per-core spmm: edges of the half (sorted by dst row) are grouped into
    groups of 128 slots spanning < R=8 local rows inside one 512-row
    window.  Group boundaries are built JOINTLY for both halves (cut when
    either half fills 128 slots) so the program structure (group count,
    per-group psum offsets) is identical across cores -> one SPMD program.
  - dma_gather pulls 128 source rows (512B bf16) per group from HBM; a PE
    matmul with a bf16 selection matrix segment-sums them into PSUM
    [128 cols x 512 dst rows] per column chunk (2 chunks of 128 cols).
    A full-window zero-matmul (start=True) initializes psum, so empty dst
    rows need no slots.
  - x1 is written back (a) transposed [256, H] bf16 for the final matmul,
    (b) row-major into the PAIR-SHARED x1sh [NPAD, 256] bf16 via
    PE-transpose + indirect scatter (row indices from input data).  A pair
    AllReduce barrier makes both halves visible, then spmm2 gathers x1
    rows from the full N range.  x2^T = psum(2*S@x1) - x0^T.
  - final: out^T[(b', o), n] = sum_m W_m^T x_m^T per 512-col chunk + bias;
    host reassembles the 8 core outputs.
"""

import hashlib
import sys

import numpy as np

sys.path.insert(0, "/opt/trn_rl_repo")

# ---------------------------------------------------------------- constants
N = 20000
B = 16
F = 64
K = 2
M = K + 1
OUT = 64
E = 640000

NCORES = 8
BPP = 4                    # batches per pair
C2 = BPP * F               # 256 columns per core
HN = N // 2                # real rows per half (10000)
WIN = 512
NW = (HN + WIN - 1) // WIN          # 20 windows per half
H = NW * WIN               # padded rows per half (10240)
NPAD = 2 * H               # padded global rows (20480)
R = 8                      # selmat width (max row span per group)
GCH = 16                   # groups per dma_gather call (2048 idxs)
DUMP0 = N                  # scatter dump row base for pad rows

PHASES = 3     # debug knob: 1 = spmm1 only, 2 = +spmm2, 3 = full

_cache = {}


# ------------------------------------------------------------- host schedule
def _build_schedule2(rows, cols, vals):
    """Joint pad/group of the (row-sorted) edge list for the two halves.

    Groups span < R local rows inside one 512-local-row window and close
    when EITHER half fills 128 slots, so both halves share g_r0/g_off/g_rg
    and the per-window group ranges.  Empty rows need no slots (psum is
    zero-initialized by a full-window matmul).
    """
    rows = np.asarray(rows)
    cols = np.asarray(cols)
    vals = np.asarray(vals)
    split = int(np.searchsorted(rows, HN))
    h_rows = [rows[:split], rows[split:] - HN]
    h_cols = [cols[:split], cols[split:]]
    h_vals = [vals[:split], vals[split:]]
    h_ptr = [np.searchsorted(h_rows[h], np.arange(HN + 1)) for h in range(2)]

    s_cols = [[], []]
    s_vals = [[], []]
    s_rowrel = [[], []]
    g_r0 = []

    cur = [128, 128]
    cur_r0 = -10 ** 9

    def close_group():
        if g_r0 and (cur[0] < 128 or cur[1] < 128):
            for h in range(2):
                pad = 128 - cur[h]
                if pad:
                    s_cols[h].append(np.zeros(pad, np.int32))
                    s_vals[h].append(np.zeros(pad, np.float32))
                    s_rowrel[h].append(np.zeros(pad, np.int8))
                cur[h] = 128

    for r in range(HN):
        lo = [h_ptr[h][r] for h in range(2)]
        hi = [h_ptr[h][r + 1] for h in range(2)]
        n_h = [int(hi[h] - lo[h]) for h in range(2)]
        if n_h[0] == 0 and n_h[1] == 0:
            continue
        pos = [0, 0]
        while pos[0] < n_h[0] or pos[1] < n_h[1]:
            full = any(cur[h] == 128 and pos[h] < n_h[h] for h in range(2))
            if (full or r >= cur_r0 + R
                    or (r // WIN) != (cur_r0 // WIN)):
                close_group()
                cur_r0 = r
                g_r0.append(r)
                cur = [0, 0]
            for h in range(2):
                take = min(128 - cur[h], n_h[h] - pos[h])
                if take > 0:
                    a = lo[h] + pos[h]
                    s_cols[h].append(h_cols[h][a:a + take].astype(np.int32))
                    s_vals[h].append(h_vals[h][a:a + take].astype(np.float32))
                    s_rowrel[h].append(np.full(take, r - cur_r0, np.int8))
                    cur[h] += take
                    pos[h] += take
    close_group()

    g_r0 = np.asarray(g_r0, np.int32)
    G = len(g_r0)
    halves = []
    for h in range(2):
        cols_pad = np.concatenate(s_cols[h])
        vals_pad = np.concatenate(s_vals[h])
        rowrel = np.concatenate(s_rowrel[h])
        assert len(cols_pad) == G * 128
        halves.append(dict(cols_pad=cols_pad, vals_pad=vals_pad,
                           rowrel=rowrel))

    g_win = g_r0 // WIN
    g_off = g_r0 - g_win * WIN
    g_rg = np.minimum(R, WIN - g_off)
    for h in range(2):
        assert (halves[h]["rowrel"] < np.repeat(g_rg, 128)).all()

    win_g0 = np.searchsorted(g_win, np.arange(NW))
    win_g1 = np.searchsorted(g_win, np.arange(NW) + 1)
    assert (win_g1 > win_g0).all(), "empty window"

    return dict(halves=halves, g_r0=g_r0, g_off=g_off, g_rg=g_rg,
                win_g0=win_g0, win_g1=win_g1, S=G * 128, G=G,
                maxwg=int((win_g1 - win_g0).max()))


def _selmat2(sched, half, scale):
    """bf16 [128, G*R]: sel[p, g*R + rr] = scaled val of slot (g*128+p)."""
    import ml_dtypes

    G = sched["G"]
    hd = sched["halves"][half]
    sel = np.zeros((G, 128, R), np.float32)
    s = np.arange(sched["S"])
    sel[s // 128, s % 128, hd["rowrel"]] = scale * hd["vals_pad"]
    return np.ascontiguousarray(
        sel.transpose(1, 0, 2).reshape(128, G * R)).astype(ml_dtypes.bfloat16)


def _idx_wrap2(cols_pad):
    """dma_gather index layout: [128, S/16] int16, idx i at (i%16, i//16),
    replicated across the 8 Q7 cores (partition groups of 16)."""
    a = cols_pad.astype(np.int16).reshape(-1, 16).T   # [16, S/16]
    return np.ascontiguousarray(np.tile(a, (8, 1)))


def _sidx(half):
    """[128, NW*4] i32 scatter rows: col (w*4+j) -> global rows of local
    chunk (w*512 + j*128 .. +128); pad local rows (>= HN) go to dump."""
    out = np.empty((128, NW * 4), np.int32)
    for w in range(NW):
        for j in range(4):
            loc = w * WIN + j * 128 + np.arange(128)
            glob = np.where(loc < HN, half * HN + loc, DUMP0 + loc % 128)
            out[:, w * 4 + j] = glob
    return out


# ------------------------------------------------------------ device program
def _build_program2(sched, nq=4, xgbufs=6):
    import concourse.bacc as bacc
    import concourse.mybir as mybir
    from concourse import bass
    from concourse.tile import TileContext

    f32 = mybir.dt.float32
    bf16 = mybir.dt.bfloat16
    i16 = mybir.dt.int16
    i32 = mybir.dt.int32

    G = sched["G"]
    S = sched["S"]
    g_off = sched["g_off"]
    g_rg = sched["g_rg"]
    win_g0 = sched["win_g0"]
    win_g1 = sched["win_g1"]
    MAXWG = sched["maxwg"]

    nc = bacc.Bacc("TRN2", target_bir_lowering=False, debug=False,
                   num_devices=NCORES, num_swdge_queues=nq,
                   dynamic_dma_scratch_size=65536)

    x0_rm = nc.declare_dram_parameter("x0_rm", [NPAD, C2], bf16, isOutput=False)
    x0T_d = nc.declare_dram_parameter("x0T", [C2, H], bf16, isOutput=False)
    idx_d = nc.declare_dram_parameter("idx", [128, S // 16], i16, isOutput=False)
    sel1_d = nc.declare_dram_parameter("sel1", [128, G * R], bf16, isOutput=False)
    sel2_d = nc.declare_dram_parameter("sel2", [128, G * R], bf16, isOutput=False)
    sidx_d = nc.declare_dram_parameter("sidx", [128, NW * 4], i32, isOutput=False)
    w2_d = nc.declare_dram_parameter("w2", [128, M * OUT], bf16, isOutput=False)
    bias_d = nc.declare_dram_parameter("bias", [64, 1], f32, isOutput=False)
    id_d = nc.declare_dram_parameter("id128", [128, 128], bf16, isOutput=False)
    out_d = nc.declare_dram_parameter("out", [C2, H], f32, isOutput=True)

    x1sh = nc.dram_tensor("x1sh", [NPAD, C2], bf16, addr_space="Shared")
    cc_in = nc.dram_tensor("cc_in", [128, 4], bf16)
    cc_out = nc.dram_tensor("cc_out", [128, 4], bf16)

    with TileContext(nc) as tc:
        with tc.tile_pool(name="const", bufs=1) as cpool, \
             tc.tile_pool(name="io", bufs=2) as iop, \
             tc.tile_pool(name="xg", bufs=4) as xgp, \
             tc.tile_pool(name="ep", bufs=3) as epp, \
             tc.tile_pool(name="fm", bufs=2) as fmp, \
             tc.tile_pool(name="px", bufs=2, space="PSUM") as pxp, \
             tc.tile_pool(name="ptr", bufs=2, space="PSUM") as ptrp, \
             tc.tile_pool(name="po", bufs=2, space="PSUM") as pop:

            ident = cpool.tile([128, 128], bf16, tag="ident")
            nc.sync.dma_start(out=ident[:], in_=id_d[:])
            w2_sb = cpool.tile([128, M * OUT], bf16, tag="w2")
            nc.sync.dma_start(out=w2_sb[:], in_=w2_d[:])
            bias_sb = cpool.tile([64, 1], f32, tag="bias")
            nc.sync.dma_start(out=bias_sb[:], in_=bias_d[:])
            sidx_sb = cpool.tile([128, NW * 4], i32, tag="sidx")
            nc.sync.dma_start(out=sidx_sb[:], in_=sidx_d[:])
            zeros_sb = cpool.tile([128, WIN], bf16, tag="zeros")
            nc.vector.memset(zeros_sb[:], 0)
            # x0T and x1T stay resident in SBUF across all phases
            x0T_sb = cpool.tile([128, 2, H], bf16, tag="x0Ts")
            for k in range(2):
                nc.sync.dma_start(out=x0T_sb[:, k, :],
                                  in_=x0T_d[k * 128:(k + 1) * 128, :])
            x1T_sb = cpool.tile([128, 2, H], bf16, tag="x1Ts")

            # ---------------- spmm pass (shared emitter) ----------------
            def spmm(src_rm, sel_d, second):
                for w in range(NW):
                    gw0, gw1 = int(win_g0[w]), int(win_g1[w])
                    psum = []
                    for k in range(2):
                        pxt = pxp.tile([128, WIN], f32, tag=f"px{k}",
                                       name=f"px{k}")
                        psum.append(pxt)
                    for k in range(2):
                        nc.tensor.matmul(
                            psum[k][:, :], zeros_sb[:, :128], zeros_sb[:, :],
                            start=True, stop=False)
                    # whole window's idxs + selmat staged once
                    ngw = gw1 - gw0
                    idx_t = iop.tile([128, MAXWG * 8], i16, tag="idx")
                    nc.sync.dma_start(
                        out=idx_t[:, :ngw * 8],
                        in_=idx_d[:, gw0 * 8:gw1 * 8])
                    sel_t = iop.tile([128, MAXWG * R], bf16, tag="sel")
                    nc.sync.dma_start(
                        out=sel_t[:, :ngw * R],
                        in_=sel_d[:, gw0 * R:gw1 * R])
                    for qi, c0 in enumerate(range(gw0, gw1, GCH)):
                        c1 = min(c0 + GCH, gw1)
                        ng = c1 - c0
                        nidx = ng * 128
                        o8 = (c0 - gw0) * 8
                        xg = xgp.tile([128, GCH, C2], bf16, tag="xg")
                        nc.gpsimd.dma_gather(
                            xg[:, :ng, :], src_rm[:],
                            idx_t[:, o8:o8 + nidx // 16],
                            nidx, nidx, C2, single_packet=False,
                            queue_num=qi % nq)
                        for j in range(ng):
                            g = c0 + j
                            off = int(g_off[g])
                            rg = int(g_rg[g])
                            jr = (g - gw0) * R
                            for k in range(2):
                                nc.tensor.matmul(
                                    psum[k][:, off:off + rg],
                                    xg[:, j, k * 128:(k + 1) * 128],
                                    sel_t[:, jr:jr + rg],
                                    start=False, stop=(g == gw1 - 1))
                    # epilogue
                    if second:
                        # x2^T = 2*S*x1^T - x0^T (vals pre-doubled), then
                        # the final matmul for this window, all from SBUF
                        x2t = []
                        for k in range(2):
                            t = epp.tile([128, WIN], bf16, tag=f"x2t{k}",
                                         name=f"x2t{k}")
                            nc.vector.tensor_sub(
                                t[:], psum[k][:],
                                x0T_sb[:, k, w * WIN:(w + 1) * WIN])
                            x2t.append(t)
                        for bp in range(BPP):
                            k, p0 = bp // 2, (bp % 2) * 64
                            cs = slice(w * WIN, (w + 1) * WIN)
                            rhss = (x0T_sb[p0:p0 + 64, k, cs],
                                    x1T_sb[p0:p0 + 64, k, cs],
                                    x2t[k][p0:p0 + 64, :])
                            pso = pop.tile([64, WIN], f32, tag="po")
                            for m in range(M):
                                nc.tensor.matmul(
                                    pso[:, :],
                                    w2_sb[p0:p0 + 64,
                                          m * OUT:(m + 1) * OUT],
                                    rhss[m],
                                    start=(m == 0),
                                    stop=(m == M - 1))
                            o_sb = fmp.tile([64, WIN], f32, tag="os")
                            nc.vector.tensor_scalar_add(o_sb[:, :],
                                                        pso[:, :],
                                                        bias_sb[:, 0:1])
                            nc.sync.dma_start(
                                out=out_d[bp * 64:(bp + 1) * 64,
                                          w * WIN:(w + 1) * WIN],
                                in_=o_sb[:])
                    else:
                        for k in range(2):
                            nc.vector.tensor_copy(
                                x1T_sb[:, k, w * WIN:(w + 1) * WIN],
                                psum[k][:])
                        # row-major into pair-shared x1sh (PE transpose +
                        # indirect scatter with data-driven row indices)
                        for j in range(WIN // 128):
                            rm = epp.tile([128, C2], bf16, tag="rm")
                            for k in range(2):
                                ptr = ptrp.tile([128, 128], bf16, tag="ptr")
                                nc.tensor.transpose(
                                    ptr[:],
                                    x1T_sb[:, k, w * WIN + j * 128:
                                           w * WIN + (j + 1) * 128],
                                    ident[:])
                                nc.vector.tensor_copy(
                                    rm[:, k * 128:(k + 1) * 128], ptr[:])
                            nc.gpsimd.indirect_dma_start(
                                out=x1sh[:],
                                out_offset=bass.IndirectOffsetOnAxis(
                                    ap=sidx_sb[:, w * 4 + j:w * 4 + j + 1],
                                    axis=0),
                                in_=rm[:],
                                in_offset=None)

            spmm(x0_rm, sel1_d, second=False)
            if PHASES >= 2:
                tc.strict_bb_all_engine_barrier()
                # pair barrier: partner's scatters are complete once it
                # enters the AllReduce (its strict barrier precedes it)
                nc.gpsimd.dma_start(out=cc_in[:], in_=zeros_sb[:, 0:4])
                nc.gpsimd.collective_compute(
                    "AllReduce",
                    mybir.AluOpType.add,
                    replica_groups=[[0, 1], [2, 3], [4, 5], [6, 7]],
                    ins=[cc_in[:]],
                    outs=[cc_out[:]],
                )
                tc.strict_bb_all_engine_barrier()
                spmm(x1sh, sel2_d, second=True)

    nc.compile()
    return nc


# ------------------------------------------------------------------- kernel
def _make_in_maps(inp, sched):
    import ml_dtypes

    bf = ml_dtypes.bfloat16
    inputs = np.asarray(inp["inputs"], np.float32)
    weight = np.asarray(inp["weight"], np.float32)
    biases = np.asarray(inp["biases"], np.float32)

    sels = [[_selmat2(sched, h, 1.0), _selmat2(sched, h, 2.0)]
            for h in range(2)]
    idxs = [_idx_wrap2(sched["halves"][h]["cols_pad"]) for h in range(2)]
    sidxs = [_sidx(h) for h in range(2)]

    w3 = weight.reshape(F, M, OUT)
    w2 = np.ascontiguousarray(np.tile(
        np.concatenate([w3[:, m, :] for m in range(M)], axis=1),
        (2, 1))).astype(bf)
    bias2 = np.ascontiguousarray(biases[:, None]).astype(np.float32)
    id128 = np.eye(128, dtype=bf)

    # x0 column slices: [N, B, F] view; pair q takes batches 4q..4q+3
    xnbf = np.transpose(inputs, (1, 0, 2))   # [N, B, F]
    in_maps = []
    for c in range(NCORES):
        q, h = c // 2, c % 2
        x0p = xnbf[:, q * BPP:(q + 1) * BPP, :].reshape(N, C2)
        x0_rm = np.zeros((NPAD, C2), bf)
        x0_rm[:N] = x0p.astype(bf)
        x0T = np.zeros((C2, H), bf)
        x0T[:, :HN] = x0p[h * HN:(h + 1) * HN].T.astype(bf)
        in_maps.append(dict(
            x0_rm=x0_rm, x0T=x0T, idx=idxs[h],
            sel1=sels[h][0], sel2=sels[h][1], sidx=sidxs[h],
            w2=w2, bias=bias2, id128=id128))
    return in_maps


def kernel(inputs, sp_rows, sp_cols, sp_vals, weight, biases):
    from concourse.bass_utils import run_bass_kernel_spmd

    sp_rows = np.asarray(sp_rows, np.int32)
    sp_cols = np.asarray(sp_cols, np.int32)
    sp_vals = np.asarray(sp_vals, np.float32)

    key = hashlib.sha256(sp_rows.tobytes() + sp_cols.tobytes()).hexdigest()
    if key not in _cache:
        sched = _build_schedule2(sp_rows, sp_cols, sp_vals)
        nc = _build_program2(sched)
        _cache[key] = (sched, nc)
    sched, nc = _cache[key]

    in_maps = _make_in_maps(
        dict(inputs=inputs, weight=weight, biases=biases), sched)

    global _last_in_maps
    _last_in_maps = in_maps
    res = run_bass_kernel_spmd(nc, in_maps, list(range(NCORES)))

    out = np.empty((B, N, OUT), np.float32)
    for c in range(NCORES):
        q, h = c // 2, c % 2
        oc = res.results[c]["out"].reshape(BPP, OUT, H)  # [b', o, nl]
        out[q * BPP:(q + 1) * BPP, h * HN:(h + 1) * HN] = (
            oc[:, :, :HN].transpose(0, 2, 1))
    return out.reshape(B * N, OUT)

